# revision 29
# baseline (speedup 1.0000x reference)
"""Trainium2 Bass kernel: multi-head attention with Toeplitz relative bias.

Problem: B=16, L=1024, F=512, H=8, D=64 ViT patch attention.
Sharding: data-parallel over batch, 2 batches per core across 8 cores.

Device-side design (per core, fully unrolled Tile program):
  - Host pre-transposes inputs to xT [F, L] (bf16); projections contract F on
    SBUF partitions.
  - qT/kT computed transposed ([fout, L]): head pair stacked on partitions
    (64 rows each); scores use K=64 matmuls with matching partition bases, so
    no zero padding or memset is needed.
  - Scores computed transposed [k, q] (k on partitions) so attn@v needs no
    transpose of the attention matrix. ACT does exp (its only job); DVE
    multiplies in the host-precomputed exp(bias) (fp16, 2x DVE mode).
  - attn@v computed TRANSPOSED: stationary = [v | ones] (ones columns are
    free: matmul cost depends only on moving cols), moving = exp tiles
    [k, q].  Output psum [128, q]: rows 0:64 = x^T (unnormalized), rows
    64:128 = softmax denominator replicated.  Normalize = DVE reciprocal
    (psum->sbuf, same base) + DVE multiply (psum in0 + sbuf in1, mixed-space
    cross-base, verified legal).  Result lands directly in xatT layout for
    the output projection -- the PE transpose phase of the previous design is
    gone.
  - Head loop is software-pipelined: window (h,b) emits scores(h,b) while
    attn@v of the previous window's head drains, so ACT (the bottleneck at
    ~1us per exp tile) never starves.
  - No max-subtraction in softmax: |scores| <~ 1.5 by construction.
"""

import sys

for _p in ("/opt/trn_rl_repo",):
    if _p not in sys.path:
        sys.path.insert(0, _p)

import numpy as np
import ml_dtypes

import concourse.bass as bass
import concourse.mybir as mybir
import concourse.tile as tile
from concourse import bacc
from concourse.bass_utils import run_bass_kernel_spmd

B, L, F, H, D = 16, 1024, 512, 8, 64
NX, NY = 32, 32
NCORES = 8
BPC = B // NCORES  # batches per core
FP32 = mybir.dt.float32
BF16 = mybir.dt.bfloat16
FP16 = mybir.dt.float16
Exp = mybir.ActivationFunctionType.Exp
Add = mybir.AluOpType.add
Mult = mybir.AluOpType.mult
Bypass = mybir.AluOpType.bypass


def _build():
    nc = bacc.Bacc("TRN2", target_bir_lowering=False, debug=False)

    xqT_d = nc.dram_tensor("xqT", [BPC, 128, 4 * L], BF16, kind="ExternalInput").ap()
    xkvT_d = nc.dram_tensor("xkvT", [BPC, 128, 4 * L], BF16, kind="ExternalInput").ap()
    Wq_d = nc.dram_tensor("Wq", [128, 4 * F], BF16, kind="ExternalInput").ap()
    Wk_d = nc.dram_tensor("Wk", [128, 4 * F], BF16, kind="ExternalInput").ap()
    Wv_d = nc.dram_tensor("Wv", [128, 4 * F], BF16, kind="ExternalInput").ap()
    Wo_d = nc.dram_tensor("Wo", [128, 4 * F], BF16, kind="ExternalInput").ap()
    bq_d = nc.dram_tensor("bq", [F], FP32, kind="ExternalInput").ap()
    bk_d = nc.dram_tensor("bk", [F], FP32, kind="ExternalInput").ap()
    bvb_d = nc.dram_tensor("bvb", [128, 2 * F], BF16, kind="ExternalInput").ap()
    bob_d = nc.dram_tensor("bob", [128, 2 * F], BF16, kind="ExternalInput").ap()
    biasT_d = nc.dram_tensor("biasT", [H, 2, 128, 4 * L], FP16, kind="ExternalInput").ap()
    out_d = nc.dram_tensor("out", [BPC, L, F], BF16, kind="ExternalOutput").ap()
    # DRAM scratch for the denominator diag-chop reciprocal (per window slot)
    scr1_d = nc.dram_tensor("scr1", [2 * H, L], FP16, kind="Internal").ap()
    scr2_d = nc.dram_tensor("scr2", [2 * H, L], FP16, kind="Internal").ap()

    with tile.TileContext(nc) as tc:
        with (
            tc.tile_pool(name="const", bufs=1) as cpool,
            tc.tile_pool(name="xin", bufs=2) as xpool,
            tc.tile_pool(name="qkv", bufs=2) as qpool,
            tc.tile_pool(name="bias", bufs=3) as bpool,
            tc.tile_pool(name="es", bufs=6) as espool,
            tc.tile_pool(name="exp", bufs=11) as epool,
            tc.tile_pool(name="xu", bufs=6) as xupool,
            tc.tile_pool(name="nrd", bufs=3) as nrdpool,
            tc.tile_pool(name="nrr", bufs=3) as nrrpool,
            tc.tile_pool(name="os", bufs=2) as ospool,
            tc.tile_pool(name="psS", bufs=3, space="PSUM") as psS,
            tc.tile_pool(name="psU", bufs=1, space="PSUM") as psU,
        ):
            # ---- constant loads (DMA) ----
            Wq_s = cpool.tile([128, 4 * F], BF16, tag="Wq")
            Wk_s = cpool.tile([128, 4 * F], BF16, tag="Wk")
            Wv_s = cpool.tile([128, 4 * F], BF16, tag="Wv")
            Wo_s = cpool.tile([128, 4 * F], BF16, tag="Wo")

            def load_w(w_s, w_d):
                nc.sync.dma_start(out=w_s[:], in_=w_d)

            # bias for head 0 staged first: needed ~9us in
            bias_tiles = {}

            def stage_bias(h):
                tiles = []
                for hh in range(2):
                    bt = bpool.tile([128, 4 * L], FP16, tag="bias")
                    nc.sync.dma_start(out=bt[:], in_=biasT_d[h, hh])
                    tiles.append(bt)
                bias_tiles[h] = tiles

            stage_bias(0)
            load_w(Wq_s, Wq_d)
            load_w(Wk_s, Wk_d)
            bq_s = cpool.tile([128, 4], FP32, tag="bq")
            bk_s = cpool.tile([128, 4], FP32, tag="bk")
            for b_s, b_d in ((bq_s, bq_d), (bk_s, bk_d)):
                nc.sync.dma_start(out=b_s[:], in_=b_d.rearrange("(c p) -> p c", p=128))

            xq, xkv = [], []
            for b in range(BPC):
                xq_t = xpool.tile([128, 4 * L], BF16, tag="xq")
                xkv_t = xpool.tile([128, 4 * L], BF16, tag="xkv")
                nc.sync.dma_start(out=xkv_t[:], in_=xkvT_d[b])
                nc.sync.dma_start(out=xq_t[:], in_=xqT_d[b])
                xq.append(xq_t)
                xkv.append(xkv_t)
            load_w(Wv_s, Wv_d)
            bvb_s = cpool.tile([128, 2 * F], BF16, tag="bvb")
            nc.sync.dma_start(out=bvb_s[:], in_=bvb_d)
            stage_bias(1)
            load_w(Wo_s, Wo_d)
            bob_s = cpool.tile([128, 2 * F], BF16, tag="bob")
            nc.sync.dma_start(out=bob_s[:], in_=bob_d)

            # ---- persistent per-batch tiles ----
            qT, kT, vAug, xatT = [], [], [], []
            for b in range(BPC):
                qT_t = qpool.tile([128, 4 * L], BF16, tag="qT")
                kT_t = qpool.tile([128, 4 * L], BF16, tag="kT")
                # vAug [128, lt(8) x h(8) x 65]: cols 0:64 v, col 64 ones
                vAug_t = qpool.tile([128, 8 * 8 * 65], FP16, tag="vAug")
                xatT_t = qpool.tile([128, 4 * L], BF16, tag="xatT")
                qT.append(qT_t)
                kT.append(kT_t)
                vAug.append(vAug_t)
                xatT.append(xatT_t)

            for b in range(BPC):
                nc.gpsimd.memset(
                    vAug[b][:].rearrange("p (t h c) -> p t h c", t=8, h=8)[
                        :, :, :, 64:65
                    ],
                    1.0,
                )

            # ---- background PE work generators (interleaved into windows) ----
            def v_proj_steps(b):
                # v natural [L, F] (+bv): per lt-pair one psS tile [128, 1024]
                for ltp in range(4):
                    pv = psS.tile([128, 1024], FP32, tag="ps")
                    for half in range(2):
                        lt = 2 * ltp + half
                        for kc in range(4):
                            nc.tensor.matmul(
                                pv[:, half * 512 : (half + 1) * 512],
                                xkv[b][:, kc * L + lt * 128 : kc * L + (lt + 1) * 128],
                                Wv_s[:, kc * F : (kc + 1) * F],
                                start=(kc == 0),
                                stop=(kc == 3),
                            )
                    # evict both halves in one DVE op (+bv broadcast), fp16
                    nc.vector.scalar_tensor_tensor(
                        vAug[b][:, ltp * 1040 : (ltp + 1) * 1040]
                        .rearrange("p (t h c) -> p t h c", t=2, h=8)[:, :, :, 0:64],
                        pv[:],
                        1.0,
                        bvb_s[:],
                        Bypass,
                        Add,
                    )
                    yield

            def qk_proj_steps(fo, b):
                # qT/kT transposed [fout, L]; head pair stacked on partitions
                for which, w_s, b_s, x_t, dst in (
                    ("q", Wq_s, bq_s, xq[b], qT[b]),
                    ("k", Wk_s, bk_s, xkv[b], kT[b]),
                ):
                    pq = psS.tile([128, 1024], FP32, tag="ps")
                    for lc in range(2):
                        for kc in range(4):
                            nc.tensor.matmul(
                                pq[:, lc * 512 : (lc + 1) * 512],
                                w_s[:, kc * F + fo * 128 : kc * F + (fo + 1) * 128],
                                x_t[:, kc * L + lc * 512 : kc * L + (lc + 1) * 512],
                                start=(kc == 0),
                                stop=(kc == 3),
                            )
                        if lc == 0:
                            yield
                    nc.vector.tensor_scalar_add(
                        dst[:, fo * L : (fo + 1) * L], pq[:], b_s[:, fo : fo + 1]
                    )
                    yield

            def phase_c_steps(b):
                # out projection: out[q, f] = sum_c xatT_c^T @ Wo_c (+bo)
                for ltp in range(4):
                    po = psS.tile([128, 1024], FP32, tag="ps")
                    for half in range(2):
                        lt = 2 * ltp + half
                        for c in range(4):
                            nc.tensor.matmul(
                                po[:, half * 512 : (half + 1) * 512],
                                xatT[b][:, c * L + lt * 128 : c * L + (lt + 1) * 128],
                                Wo_s[:, c * F : (c + 1) * F],
                                start=(c == 0),
                                stop=(c == 3),
                            )
                        yield
                    os_t = ospool.tile([128, 1024], BF16, tag="os")
                    nc.vector.scalar_tensor_tensor(
                        os_t[:], po[:], 1.0, bob_s[:], Bypass, Add
                    )
                    nc.sync.dma_start(
                        out=out_d[b, ltp * 256 : (ltp + 1) * 256, :].rearrange(
                            "(t p) f -> p t f", t=2
                        ),
                        in_=os_t[:].rearrange("p (t f) -> p t f", t=2),
                    )
                    yield

            # background queue: list of generators, consumed a few steps/slot
            bg = []

            def bg_step(n=1):
                for _ in range(n):
                    while bg:
                        try:
                            next(bg[0])
                            break
                        except StopIteration:
                            bg.pop(0)
                    else:
                        return

            # ---- window schedule ----
            # window w (w = 0..15): scores/exp for (h, b) = (w//2, w%2),
            # attn@v for the previous window's (h, b).
            # qk fo0/b0 emitted fully up front (window 0 needs it);
            # qk fo0/b1 + v b0 + v b1 drain during window 0 (needed at
            # windows 1, 1, 2 respectively).
            for _ in qk_proj_steps(0, 0):
                pass
            bg.append(qk_proj_steps(0, 1))
            bg.append(v_proj_steps(0))
            bg.append(v_proj_steps(1))
            # remaining projections: fo chunk f first used at window 4f
            pending_proj = {1: (1, 0), 2: (1, 1), 5: (2, 0), 6: (2, 1),
                            9: (3, 0), 10: (3, 1)}

            ex_tiles = {}  # (b, kt) -> ex tile for the in-flight head
            prev = None  # (h, b, psU_tile) of the window being drained

            def emit_attnv_slot(h, b, pu, kt):
                for qc in range(2):
                    nc.tensor.matmul(
                        pu[0:65, qc * 512 : (qc + 1) * 512],
                        vAug[b][:, (kt * 8 + h) * 65 : (kt * 8 + h + 1) * 65],
                        ex_tiles[(b, kt)][:, qc * 512 : (qc + 1) * 512],
                        start=(kt == 0),
                        stop=(kt == 7),
                    )

            # ---- staggered softmax-normalize pipeline ----
            # psum rows 0:64 = x^T (unnormalized), row 64 = denominators.
            # Stage A (window end): one DVE copy evicts x^T + denom row to
            #   SBUF fp16, freeing the psU banks; two DMAs chop the denom row
            #   into [64, 16] via DRAM so the slow per-lane DVE reciprocal
            #   costs ~16 elements instead of 1024.
            # Stage B (one window later, inputs already landed): reciprocal,
            #   DMA merge back to a row, gpsimd partition-broadcast.
            # Stage C (two windows later): one fp16 SBUF x SBUF multiply into
            #   the xatT layout the output projection consumes.
            # The stagger keeps every in-order queue from head-of-line
            # blocking on the DMA round trips.
            norm_state = {}

            def norm_a(w, pu):
                xu_t = xupool.tile([128, 1024], FP16, tag="xu")
                with nc.allow_low_precision(
                    reason="unnormalized attn output held in fp16; <1e-3 rel"
                ):
                    nc.vector.tensor_copy(xu_t[0:65, :], pu[0:65, :])
                nc.sync.dma_start(
                    out=scr1_d[w].rearrange("(p n) -> p n", p=1), in_=xu_t[64:65, :]
                )
                diag_t = nrdpool.tile([128, 16], FP16, tag="diag")
                nc.sync.dma_start(
                    out=diag_t[0:64, :], in_=scr1_d[w].rearrange("(a b) -> a b", a=64)
                )
                norm_state[w] = [xu_t, diag_t]

            def norm_b(w):
                xu_t, diag_t = norm_state[w]
                rcd_t = nrdpool.tile([128, 16], FP16, tag="rcd")
                with nc.allow_low_precision(
                    reason="softmax denominator reciprocal in fp16; <1e-3 rel"
                ):
                    nc.vector.reciprocal(rcd_t[0:64, :], diag_t[0:64, :])
                nc.sync.dma_start(
                    out=scr2_d[w].rearrange("(a b) -> a b", a=64), in_=rcd_t[0:64, :]
                )
                rrow_t = nrrpool.tile([128, 1024], FP16, tag="rrow")
                nc.sync.dma_start(
                    out=rrow_t[0:1, :], in_=scr2_d[w].rearrange("(p n) -> p n", p=1)
                )
                norm_state[w] = [xu_t, rrow_t]

            def norm_b2(w):
                xu_t, rrow_t = norm_state[w]
                rrep_t = nrrpool.tile([128, 1024], FP16, tag="rrep")
                nc.gpsimd.partition_broadcast(rrep_t[0:64, :], rrow_t[0:1, :])
                norm_state[w] = [xu_t, rrep_t]

            def norm_c(w):
                xu_t, rrep_t = norm_state.pop(w)
                h, b = w // 2, w % 2
                hp = (h % 2) * 64
                c = h // 2
                with nc.allow_low_precision(
                    reason="softmax normalize multiply in fp16; <1e-3 rel"
                ):
                    # gpsimd: off the DVE critical path; inputs landed stages ago
                    nc.gpsimd.tensor_tensor(
                        xatT[b][hp : hp + 64, c * L : (c + 1) * L],
                        xu_t[0:64, :],
                        rrep_t[0:64, :],
                        Mult,
                    )

            for w in range(2 * H):
                h, b = w // 2, w % 2
                hp = (h % 2) * 64
                hc = (h // 2) * L
                if w in pending_proj:
                    fo, pb = pending_proj[w]
                    bg.append(qk_proj_steps(fo, pb))
                if b == 0 and h + 1 < H:
                    stage_bias(h + 1)
                cur = psU.tile([128, 1024], FP32, tag="u")
                for kt in range(8):
                    ps = psS.tile([128, 1024], FP32, tag="ps")
                    for qc in range(2):
                        nc.tensor.matmul(
                            ps[:, qc * 512 : (qc + 1) * 512],
                            kT[b][hp : hp + 64, hc + kt * 128 : hc + (kt + 1) * 128],
                            qT[b][hp : hp + 64, hc + qc * 512 : hc + (qc + 1) * 512],
                            start=True,
                            stop=True,
                        )
                    if prev is not None:
                        emit_attnv_slot(prev[0], prev[1], prev[2], kt)
                    es = espool.tile([128, 1024], FP16, tag="es")
                    nc.scalar.activation(es[:], ps[:], Exp)
                    ex = epool.tile([128, 1024], FP16, tag="ex")
                    nc.vector.tensor_tensor(
                        ex[:],
                        es[:],
                        bias_tiles[h][kt // 4][:, (kt % 4) * L : (kt % 4 + 1) * L],
                        Mult,
                    )
                    ex_tiles[(b, kt)] = ex
                    bg_step(2)
                # staggered norm stages: A for the pair just drained (w-1),
                # B for w-2, B2 (broadcast) for w-3, C (multiply) for w-4
                if prev is not None:
                    norm_a(w - 1, prev[2])
                if w >= 2:
                    norm_b(w - 2)
                if w >= 3:
                    norm_b2(w - 3)
                if w >= 4:
                    norm_c(w - 4)
                prev = (h, b, cur)

            # drain: attn@v for the last window pair (h=H-1, b=1) = w 15
            for kt in range(8):
                emit_attnv_slot(prev[0], prev[1], prev[2], kt)
                bg_step(2)
            norm_a(15, prev[2])
            norm_b(14)
            norm_b2(13)
            norm_c(12)
            norm_b(15)
            norm_b2(14)
            norm_c(13)
            norm_b2(15)
            norm_c(14)
            bg.append(phase_c_steps(0))
            bg_step(4)
            norm_c(15)
            bg.append(phase_c_steps(1))
            while bg:
                bg_step()

    nc.compile()
    return nc


_NC = None


def _get_nc():
    global _NC
    if _NC is None:
        _NC = _build()
    return _NC


def _prep_in_maps(inputs):
    bf16 = ml_dtypes.bfloat16
    xq = np.asarray(inputs["inputs_q"], dtype=np.float32)
    xkv = np.asarray(inputs["inputs_kv"], dtype=np.float32)
    Wq = (np.asarray(inputs["Wq"], dtype=np.float32) * 0.125).astype(bf16)
    bq = np.asarray(inputs["bq"], dtype=np.float32) * 0.125
    Wk = np.asarray(inputs["Wk"], dtype=np.float32).astype(bf16)
    bk = np.asarray(inputs["bk"], dtype=np.float32)
    Wv = np.asarray(inputs["Wv"], dtype=np.float32).astype(bf16)
    Wo = np.asarray(inputs["Wo"], dtype=np.float32).astype(bf16)
    bv = np.asarray(inputs["bv"], dtype=np.float32)
    bo = np.asarray(inputs["bo"], dtype=np.float32)
    bvb = np.tile(bv[None, :], (128, 2)).astype(bf16)
    bob = np.tile(bo[None, :], (128, 2)).astype(bf16)
    toe = np.asarray(inputs["toeplitz"], dtype=np.float32)

    def pack_x(x):
        # [B, L, F] -> xT [B, F, L] -> p-major packed [B, 128, 4*L]
        xT = x.transpose(0, 2, 1).reshape(B, 4, 128, L)
        return np.ascontiguousarray(xT.transpose(0, 2, 1, 3)).reshape(B, 128, 4 * L).astype(bf16)

    def pack_w(w):
        # [F, F] -> p-major packed [128, 4*F]
        return np.ascontiguousarray(
            w.reshape(4, 128, F).transpose(1, 0, 2)
        ).reshape(128, 4 * F)

    xqT = pack_x(xq)
    xkvT = pack_x(xkv)
    Wq, Wk, Wv, Wo = (pack_w(np.asarray(w, dtype=bf16)) for w in (Wq, Wk, Wv, Wo))

    coords = np.arange(L)
    xi, yi = coords // NY, coords % NY
    dx = xi[:, None] - xi[None, :] + NX
    dy = yi[:, None] - yi[None, :] + NY
    idx = dx * (2 * NY) + dy  # [L(q), L(k)]
    bias = toe[:, idx]  # [H, L(q), L(k)]
    biasT = np.exp(bias.transpose(0, 2, 1))  # [H, L(k), L(q)]
    # p-major packed [H, 2, 128, 4*L]: biasP[h, hh, p, t, :] = biasT[h, hh*512+t*128+p]
    biasT = np.ascontiguousarray(
        biasT.reshape(H, 2, 4, 128, L).transpose(0, 1, 3, 2, 4)
    ).reshape(H, 2, 128, 4 * L).astype(np.float16)

    in_maps = []
    for i in range(NCORES):
        sl = slice(i * BPC, (i + 1) * BPC)
        in_maps.append(
            {
                "xqT": np.ascontiguousarray(xqT[sl]),
                "xkvT": np.ascontiguousarray(xkvT[sl]),
                "Wq": Wq, "Wk": Wk, "Wv": Wv, "Wo": Wo,
                "bq": bq, "bk": bk, "bvb": bvb, "bob": bob,
                "biasT": biasT,
            }
        )
    return in_maps


def _run(inputs, trace=False):
    from concourse.bass_interp import get_hw_module

    nc = _get_nc()
    in_maps = _prep_in_maps(inputs)
    old_m = nc.m
    nc.m = get_hw_module(nc.m)
    try:
        res = run_bass_kernel_spmd(
            nc, in_maps, core_ids=list(range(NCORES)), trace=trace
        )
    finally:
        nc.m = old_m
    out = np.concatenate(
        [np.asarray(r["out"], dtype=np.float32) for r in res.results], axis=0
    )  # [B, L, F]
    return out.reshape(B, L, H, D), res


def kernel(**inputs) -> np.ndarray:
    out, _ = _run(inputs, trace=False)
    return out


# revision 31
# speedup vs baseline: 1.0180x; 1.0180x over previous
"""Trainium2 Bass kernel: multi-head attention with Toeplitz relative bias.

Problem: B=16, L=1024, F=512, H=8, D=64 ViT patch attention.
Sharding: data-parallel over batch, 2 batches per core across 8 cores.

Device-side design (per core, fully unrolled Tile program):
  - Host pre-transposes inputs to xT [F, L] (bf16); projections contract F on
    SBUF partitions.
  - qT/kT computed transposed ([fout, L]): head pair stacked on partitions
    (64 rows each); scores use K=64 matmuls with matching partition bases, so
    no zero padding or memset is needed.
  - Scores computed transposed [k, q] (k on partitions) so attn@v needs no
    transpose of the attention matrix. ACT does exp (its only job); DVE
    multiplies in the host-precomputed exp(bias) (fp16, 2x DVE mode).
  - attn@v computed TRANSPOSED: stationary = [v | ones] (ones columns are
    free: matmul cost depends only on moving cols), moving = exp tiles
    [k, q].  Output psum [128, q]: rows 0:64 = x^T (unnormalized), rows
    64:128 = softmax denominator replicated.  Normalize = DVE reciprocal
    (psum->sbuf, same base) + DVE multiply (psum in0 + sbuf in1, mixed-space
    cross-base, verified legal).  Result lands directly in xatT layout for
    the output projection -- the PE transpose phase of the previous design is
    gone.
  - Head loop is software-pipelined: window (h,b) emits scores(h,b) while
    attn@v of the previous window's head drains, so ACT (the bottleneck at
    ~1us per exp tile) never starves.
  - No max-subtraction in softmax: |scores| <~ 1.5 by construction.
"""

import sys

for _p in ("/opt/trn_rl_repo",):
    if _p not in sys.path:
        sys.path.insert(0, _p)

import numpy as np
import ml_dtypes

import concourse.bass as bass
import concourse.mybir as mybir
import concourse.tile as tile
from concourse import bacc
from concourse.bass_utils import run_bass_kernel_spmd

B, L, F, H, D = 16, 1024, 512, 8, 64
NX, NY = 32, 32
NCORES = 8
BPC = B // NCORES  # batches per core
FP32 = mybir.dt.float32
BF16 = mybir.dt.bfloat16
FP16 = mybir.dt.float16
Exp = mybir.ActivationFunctionType.Exp
Add = mybir.AluOpType.add
Mult = mybir.AluOpType.mult
Bypass = mybir.AluOpType.bypass


def _build():
    nc = bacc.Bacc("TRN2", target_bir_lowering=False, debug=False)

    xqT_d = nc.dram_tensor("xqT", [BPC, 128, 4 * L], BF16, kind="ExternalInput").ap()
    xkvT_d = nc.dram_tensor("xkvT", [BPC, 128, 4 * L], BF16, kind="ExternalInput").ap()
    Wq_d = nc.dram_tensor("Wq", [128, 4 * F], BF16, kind="ExternalInput").ap()
    Wk_d = nc.dram_tensor("Wk", [128, 4 * F], BF16, kind="ExternalInput").ap()
    Wv_d = nc.dram_tensor("Wv", [128, 4 * F], BF16, kind="ExternalInput").ap()
    Wo_d = nc.dram_tensor("Wo", [128, 4 * F], BF16, kind="ExternalInput").ap()
    bq_d = nc.dram_tensor("bq", [F], FP32, kind="ExternalInput").ap()
    bk_d = nc.dram_tensor("bk", [F], FP32, kind="ExternalInput").ap()
    bvb_d = nc.dram_tensor("bvb", [128, 2 * F], BF16, kind="ExternalInput").ap()
    bob_d = nc.dram_tensor("bob", [128, 2 * F], BF16, kind="ExternalInput").ap()
    biasT_d = nc.dram_tensor("biasT", [H, 2, 128, 4 * L], FP16, kind="ExternalInput").ap()
    out_d = nc.dram_tensor("out", [BPC, L, F], BF16, kind="ExternalOutput").ap()
    # DRAM scratch for the denominator diag-chop reciprocal (per window slot)
    scr1_d = nc.dram_tensor("scr1", [2 * H, L], FP16, kind="Internal").ap()
    scr2_d = nc.dram_tensor("scr2", [2 * H, L], FP16, kind="Internal").ap()

    with tile.TileContext(nc) as tc:
        with (
            tc.tile_pool(name="const", bufs=1) as cpool,
            tc.tile_pool(name="xin", bufs=2) as xpool,
            tc.tile_pool(name="qkv", bufs=2) as qpool,
            tc.tile_pool(name="bias", bufs=3) as bpool,
            tc.tile_pool(name="es", bufs=6) as espool,
            tc.tile_pool(name="exp", bufs=11) as epool,
            tc.tile_pool(name="xu", bufs=6) as xupool,
            tc.tile_pool(name="nrd", bufs=3) as nrdpool,
            tc.tile_pool(name="nrr", bufs=3) as nrrpool,
            tc.tile_pool(name="os", bufs=2) as ospool,
            tc.tile_pool(name="psS", bufs=3, space="PSUM") as psS,
            tc.tile_pool(name="psU", bufs=1, space="PSUM") as psU,
        ):
            # ---- constant loads (DMA) ----
            Wq_s = cpool.tile([128, 4 * F], BF16, tag="Wq")
            Wk_s = cpool.tile([128, 4 * F], BF16, tag="Wk")
            Wv_s = cpool.tile([128, 4 * F], BF16, tag="Wv")
            Wo_s = cpool.tile([128, 4 * F], BF16, tag="Wo")

            def load_w(w_s, w_d):
                nc.sync.dma_start(out=w_s[:], in_=w_d)

            # bias for head 0 staged first: needed ~9us in
            bias_tiles = {}

            def stage_bias(h):
                tiles = []
                for hh in range(2):
                    bt = bpool.tile([128, 4 * L], FP16, tag="bias")
                    nc.sync.dma_start(out=bt[:], in_=biasT_d[h, hh])
                    tiles.append(bt)
                bias_tiles[h] = tiles

            stage_bias(0)
            load_w(Wq_s, Wq_d)
            load_w(Wk_s, Wk_d)
            bq_s = cpool.tile([128, 4], FP32, tag="bq")
            bk_s = cpool.tile([128, 4], FP32, tag="bk")
            for b_s, b_d in ((bq_s, bq_d), (bk_s, bk_d)):
                nc.sync.dma_start(out=b_s[:], in_=b_d.rearrange("(c p) -> p c", p=128))

            xq, xkv = [], []
            for b in range(BPC):
                xq_t = xpool.tile([128, 4 * L], BF16, tag="xq")
                xkv_t = xpool.tile([128, 4 * L], BF16, tag="xkv")
                nc.sync.dma_start(out=xkv_t[:], in_=xkvT_d[b])
                nc.sync.dma_start(out=xq_t[:], in_=xqT_d[b])
                xq.append(xq_t)
                xkv.append(xkv_t)
            load_w(Wv_s, Wv_d)
            bvb_s = cpool.tile([128, 2 * F], BF16, tag="bvb")
            nc.sync.dma_start(out=bvb_s[:], in_=bvb_d)
            stage_bias(1)
            load_w(Wo_s, Wo_d)
            bob_s = cpool.tile([128, 2 * F], BF16, tag="bob")
            nc.sync.dma_start(out=bob_s[:], in_=bob_d)

            # ---- persistent per-batch tiles ----
            qT, kT, vAug, xatT = [], [], [], []
            for b in range(BPC):
                qT_t = qpool.tile([128, 4 * L], BF16, tag="qT")
                kT_t = qpool.tile([128, 4 * L], BF16, tag="kT")
                # vAug [128, lt(8) x h(8) x 65]: cols 0:64 v, col 64 ones
                vAug_t = qpool.tile([128, 8 * 8 * 65], FP16, tag="vAug")
                xatT_t = qpool.tile([128, 4 * L], BF16, tag="xatT")
                qT.append(qT_t)
                kT.append(kT_t)
                vAug.append(vAug_t)
                xatT.append(xatT_t)

            for b in range(BPC):
                nc.gpsimd.memset(
                    vAug[b][:].rearrange("p (t h c) -> p t h c", t=8, h=8)[
                        :, :, :, 64:65
                    ],
                    1.0,
                )

            # ---- background PE work generators (interleaved into windows) ----
            def v_proj_steps(b):
                # v natural [L, F] (+bv): per lt-pair one psS tile [128, 1024]
                for ltp in range(4):
                    pv = psS.tile([128, 1024], FP32, tag="ps")
                    for half in range(2):
                        lt = 2 * ltp + half
                        for kc in range(4):
                            nc.tensor.matmul(
                                pv[:, half * 512 : (half + 1) * 512],
                                xkv[b][:, kc * L + lt * 128 : kc * L + (lt + 1) * 128],
                                Wv_s[:, kc * F : (kc + 1) * F],
                                start=(kc == 0),
                                stop=(kc == 3),
                            )
                    # evict both halves in one DVE op (+bv broadcast), fp16
                    nc.vector.scalar_tensor_tensor(
                        vAug[b][:, ltp * 1040 : (ltp + 1) * 1040]
                        .rearrange("p (t h c) -> p t h c", t=2, h=8)[:, :, :, 0:64],
                        pv[:],
                        1.0,
                        bvb_s[:],
                        Bypass,
                        Add,
                    )
                    yield

            def qk_proj_steps(fo, b):
                # qT/kT transposed [fout, L]; head pair stacked on partitions
                for which, w_s, b_s, x_t, dst in (
                    ("q", Wq_s, bq_s, xq[b], qT[b]),
                    ("k", Wk_s, bk_s, xkv[b], kT[b]),
                ):
                    pq = psS.tile([128, 1024], FP32, tag="ps")
                    for lc in range(2):
                        for kc in range(4):
                            nc.tensor.matmul(
                                pq[:, lc * 512 : (lc + 1) * 512],
                                w_s[:, kc * F + fo * 128 : kc * F + (fo + 1) * 128],
                                x_t[:, kc * L + lc * 512 : kc * L + (lc + 1) * 512],
                                start=(kc == 0),
                                stop=(kc == 3),
                            )
                        if lc == 0:
                            yield
                    nc.vector.tensor_scalar_add(
                        dst[:, fo * L : (fo + 1) * L], pq[:], b_s[:, fo : fo + 1]
                    )
                    yield

            def phase_c_steps(b):
                # out projection: out[q, f] = sum_c xatT_c^T @ Wo_c (+bo)
                for ltp in range(4):
                    po = psS.tile([128, 1024], FP32, tag="ps")
                    for half in range(2):
                        lt = 2 * ltp + half
                        for c in range(4):
                            nc.tensor.matmul(
                                po[:, half * 512 : (half + 1) * 512],
                                xatT[b][:, c * L + lt * 128 : c * L + (lt + 1) * 128],
                                Wo_s[:, c * F : (c + 1) * F],
                                start=(c == 0),
                                stop=(c == 3),
                            )
                        yield
                    os_t = ospool.tile([128, 1024], BF16, tag="os")
                    nc.vector.scalar_tensor_tensor(
                        os_t[:], po[:], 1.0, bob_s[:], Bypass, Add
                    )
                    nc.sync.dma_start(
                        out=out_d[b, ltp * 256 : (ltp + 1) * 256, :].rearrange(
                            "(t p) f -> p t f", t=2
                        ),
                        in_=os_t[:].rearrange("p (t f) -> p t f", t=2),
                    )
                    yield

            # background queue: list of generators, consumed a few steps/slot
            bg = []

            def bg_step(n=1):
                for _ in range(n):
                    while bg:
                        try:
                            next(bg[0])
                            break
                        except StopIteration:
                            bg.pop(0)
                    else:
                        return

            # ---- window schedule ----
            # window w (w = 0..15): scores/exp for (h, b) = (w//2, w%2),
            # attn@v for the previous window's (h, b).
            # qk fo0/b0 emitted fully up front (window 0 needs it);
            # qk fo0/b1 + v b0 + v b1 drain during window 0 (needed at
            # windows 1, 1, 2 respectively).
            for _ in qk_proj_steps(0, 0):
                pass
            bg.append(qk_proj_steps(0, 1))
            bg.append(v_proj_steps(0))
            bg.append(v_proj_steps(1))
            # remaining projections: fo chunk f first used at window 4f
            pending_proj = {1: (1, 0), 2: (1, 1), 5: (2, 0), 6: (2, 1),
                            9: (3, 0), 10: (3, 1)}

            ex_tiles = {}  # (b, kt) -> ex tile for the in-flight head
            prev = None  # (h, b, psU_tile) of the window being drained

            def emit_attnv_slot(h, b, pu, kt):
                for qc in range(2):
                    nc.tensor.matmul(
                        pu[0:65, qc * 512 : (qc + 1) * 512],
                        vAug[b][:, (kt * 8 + h) * 65 : (kt * 8 + h + 1) * 65],
                        ex_tiles[(b, kt)][:, qc * 512 : (qc + 1) * 512],
                        start=(kt == 0),
                        stop=(kt == 7),
                    )

            # ---- staggered softmax-normalize pipeline ----
            # psum rows 0:64 = x^T (unnormalized), row 64 = denominators.
            # Stage A (window end): one DVE copy evicts x^T + denom row to
            #   SBUF fp16, freeing the psU banks; two DMAs chop the denom row
            #   into [64, 16] via DRAM so the slow per-lane DVE reciprocal
            #   costs ~16 elements instead of 1024.
            # Stage B (one window later, inputs already landed): reciprocal,
            #   DMA merge back to a row, gpsimd partition-broadcast.
            # Stage C (two windows later): one fp16 SBUF x SBUF multiply into
            #   the xatT layout the output projection consumes.
            # The stagger keeps every in-order queue from head-of-line
            # blocking on the DMA round trips.
            norm_state = {}

            def norm_a(w, pu):
                xu_t = xupool.tile([128, 1024], FP16, tag="xu")
                with nc.allow_low_precision(
                    reason="unnormalized attn output held in fp16; <1e-3 rel"
                ):
                    nc.vector.tensor_copy(xu_t[0:65, :], pu[0:65, :])
                nc.sync.dma_start(
                    out=scr1_d[w].rearrange("(p n) -> p n", p=1), in_=xu_t[64:65, :]
                )
                diag_t = nrdpool.tile([128, 16], FP16, tag="diag")
                nc.sync.dma_start(
                    out=diag_t[0:64, :], in_=scr1_d[w].rearrange("(a b) -> a b", a=64)
                )
                norm_state[w] = [xu_t, diag_t]

            def norm_b(w):
                xu_t, diag_t = norm_state[w]
                rcd_t = nrdpool.tile([128, 16], FP16, tag="rcd")
                with nc.allow_low_precision(
                    reason="softmax denominator reciprocal in fp16; <1e-3 rel"
                ):
                    nc.vector.reciprocal(rcd_t[0:64, :], diag_t[0:64, :])
                nc.sync.dma_start(
                    out=scr2_d[w].rearrange("(a b) -> a b", a=64), in_=rcd_t[0:64, :]
                )
                rrow_t = nrrpool.tile([128, 1024], FP16, tag="rrow")
                nc.sync.dma_start(
                    out=rrow_t[0:1, :], in_=scr2_d[w].rearrange("(p n) -> p n", p=1)
                )
                norm_state[w] = [xu_t, rrow_t]

            def norm_b2(w):
                xu_t, rrow_t = norm_state[w]
                rrep_t = nrrpool.tile([128, 1024], FP16, tag="rrep")
                nc.gpsimd.partition_broadcast(rrep_t[0:64, :], rrow_t[0:1, :])
                norm_state[w] = [xu_t, rrep_t]

            def norm_c(w):
                xu_t, rrep_t = norm_state.pop(w)
                h, b = w // 2, w % 2
                hp = (h % 2) * 64
                c = h // 2
                with nc.allow_low_precision(
                    reason="softmax normalize multiply in fp16; <1e-3 rel"
                ):
                    # gpsimd: off the DVE critical path; inputs landed stages ago
                    nc.gpsimd.tensor_tensor(
                        xatT[b][hp : hp + 64, c * L : (c + 1) * L],
                        xu_t[0:64, :],
                        rrep_t[0:64, :],
                        Mult,
                    )

            for w in range(2 * H):
                h, b = w // 2, w % 2
                hp = (h % 2) * 64
                hc = (h // 2) * L
                if w in pending_proj:
                    fo, pb = pending_proj[w]
                    bg.append(qk_proj_steps(fo, pb))
                if b == 0 and h + 1 < H:
                    stage_bias(h + 1)
                cur = psU.tile([128, 1024], FP32, tag="u")
                # attn@v for the previous pair runs as ONE uninterrupted PE
                # burst (all its ex tiles already exist): long continuous PE
                # stretches keep the Tensor engine at its max p-state.
                if prev is not None:
                    for kt in range(8):
                        emit_attnv_slot(prev[0], prev[1], prev[2], kt)
                        if kt % 2 == 1:
                            bg_step()
                    norm_a(w - 1, prev[2])
                for kt in range(8):
                    ps = psS.tile([128, 1024], FP32, tag="ps")
                    for qc in range(2):
                        nc.tensor.matmul(
                            ps[:, qc * 512 : (qc + 1) * 512],
                            kT[b][hp : hp + 64, hc + kt * 128 : hc + (kt + 1) * 128],
                            qT[b][hp : hp + 64, hc + qc * 512 : hc + (qc + 1) * 512],
                            start=True,
                            stop=True,
                        )
                    es = espool.tile([128, 1024], FP16, tag="es")
                    nc.scalar.activation(es[:], ps[:], Exp)
                    ex = epool.tile([128, 1024], FP16, tag="ex")
                    nc.vector.tensor_tensor(
                        ex[:],
                        es[:],
                        bias_tiles[h][kt // 4][:, (kt % 4) * L : (kt % 4 + 1) * L],
                        Mult,
                    )
                    ex_tiles[(b, kt)] = ex
                    bg_step(2)
                # staggered lazy norm stages (A emitted after the attn@v burst)
                if w >= 2:
                    norm_b(w - 2)
                if w >= 3:
                    norm_b2(w - 3)
                if w >= 4:
                    norm_c(w - 4)
                prev = (h, b, cur)

            # drain: attn@v for the last window pair (h=H-1, b=1) = w 15
            for kt in range(8):
                emit_attnv_slot(prev[0], prev[1], prev[2], kt)
                bg_step(2)
            norm_a(15, prev[2])
            norm_b(14)
            norm_b2(13)
            norm_c(12)
            norm_b(15)
            norm_b2(14)
            norm_c(13)
            norm_b2(15)
            norm_c(14)
            bg.append(phase_c_steps(0))
            bg_step(4)
            norm_c(15)
            bg.append(phase_c_steps(1))
            while bg:
                bg_step()

    nc.compile()
    return nc


_NC = None


def _get_nc():
    global _NC
    if _NC is None:
        _NC = _build()
    return _NC


def _prep_in_maps(inputs):
    bf16 = ml_dtypes.bfloat16
    xq = np.asarray(inputs["inputs_q"], dtype=np.float32)
    xkv = np.asarray(inputs["inputs_kv"], dtype=np.float32)
    Wq = (np.asarray(inputs["Wq"], dtype=np.float32) * 0.125).astype(bf16)
    bq = np.asarray(inputs["bq"], dtype=np.float32) * 0.125
    Wk = np.asarray(inputs["Wk"], dtype=np.float32).astype(bf16)
    bk = np.asarray(inputs["bk"], dtype=np.float32)
    Wv = np.asarray(inputs["Wv"], dtype=np.float32).astype(bf16)
    Wo = np.asarray(inputs["Wo"], dtype=np.float32).astype(bf16)
    bv = np.asarray(inputs["bv"], dtype=np.float32)
    bo = np.asarray(inputs["bo"], dtype=np.float32)
    bvb = np.tile(bv[None, :], (128, 2)).astype(bf16)
    bob = np.tile(bo[None, :], (128, 2)).astype(bf16)
    toe = np.asarray(inputs["toeplitz"], dtype=np.float32)

    def pack_x(x):
        # [B, L, F] -> xT [B, F, L] -> p-major packed [B, 128, 4*L]
        xT = x.transpose(0, 2, 1).reshape(B, 4, 128, L)
        return np.ascontiguousarray(xT.transpose(0, 2, 1, 3)).reshape(B, 128, 4 * L).astype(bf16)

    def pack_w(w):
        # [F, F] -> p-major packed [128, 4*F]
        return np.ascontiguousarray(
            w.reshape(4, 128, F).transpose(1, 0, 2)
        ).reshape(128, 4 * F)

    xqT = pack_x(xq)
    xkvT = pack_x(xkv)
    Wq, Wk, Wv, Wo = (pack_w(np.asarray(w, dtype=bf16)) for w in (Wq, Wk, Wv, Wo))

    coords = np.arange(L)
    xi, yi = coords // NY, coords % NY
    dx = xi[:, None] - xi[None, :] + NX
    dy = yi[:, None] - yi[None, :] + NY
    idx = dx * (2 * NY) + dy  # [L(q), L(k)]
    bias = toe[:, idx]  # [H, L(q), L(k)]
    biasT = np.exp(bias.transpose(0, 2, 1))  # [H, L(k), L(q)]
    # p-major packed [H, 2, 128, 4*L]: biasP[h, hh, p, t, :] = biasT[h, hh*512+t*128+p]
    biasT = np.ascontiguousarray(
        biasT.reshape(H, 2, 4, 128, L).transpose(0, 1, 3, 2, 4)
    ).reshape(H, 2, 128, 4 * L).astype(np.float16)

    in_maps = []
    for i in range(NCORES):
        sl = slice(i * BPC, (i + 1) * BPC)
        in_maps.append(
            {
                "xqT": np.ascontiguousarray(xqT[sl]),
                "xkvT": np.ascontiguousarray(xkvT[sl]),
                "Wq": Wq, "Wk": Wk, "Wv": Wv, "Wo": Wo,
                "bq": bq, "bk": bk, "bvb": bvb, "bob": bob,
                "biasT": biasT,
            }
        )
    return in_maps


def _run(inputs, trace=False):
    from concourse.bass_interp import get_hw_module

    nc = _get_nc()
    in_maps = _prep_in_maps(inputs)
    old_m = nc.m
    nc.m = get_hw_module(nc.m)
    try:
        res = run_bass_kernel_spmd(
            nc, in_maps, core_ids=list(range(NCORES)), trace=trace
        )
    finally:
        nc.m = old_m
    out = np.concatenate(
        [np.asarray(r["out"], dtype=np.float32) for r in res.results], axis=0
    )  # [B, L, F]
    return out.reshape(B, L, H, D), res


def kernel(**inputs) -> np.ndarray:
    out, _ = _run(inputs, trace=False)
    return out


# revision 35
# speedup vs baseline: 1.3726x; 1.3484x over previous
"""Trainium2 Bass kernel: multi-head attention with Toeplitz relative bias.

Problem: B=16, L=1024, F=512, H=8, D=64 ViT patch attention.
Sharding: data-parallel over batch, 2 batches per core across 8 cores.

Device-side design (per core, fully unrolled Tile program):
  - Host pre-packs every DRAM operand partition-major so each DMA descriptor
    covers a full 4-8KB partition row (small-descriptor overhead dominated
    the staging cost otherwise).
  - qT/kT computed transposed ([fout, L], W stationary); head pair stacked
    on partitions (64 rows each); scores use K=64 matmuls with matching
    partition bases, so kT needs no zero padding or memset.
  - v computed natural ([L, fout], xT stationary, bv via ones-row matmul).
  - Scores computed transposed [k, q] (k on partitions) so attn@v needs no
    transpose of the attention matrix. ACT does exp; DVE multiplies in the
    host-precomputed exp(bias) (fp16, 2x DVE mode).
  - attn@v in natural [q, d] layout with exp chunks as the stationary
    operand; softmax denominators accumulate into column 64 via a
    ones-column in vA, so normalization is a per-partition divide fused into
    one DVE tensor_scalar op.
  - The head loop is software-pipelined at (head, batch)-window granularity:
    window w computes scores/exp for pair (h,b) = (w//2, w%2) while the
    PREVIOUS pair's attn@v drains on the PE, so ACT (the ~1.1us/tile exp
    engine) and the PE both stay continuously busy.  QK projections are fed
    through a background queue one 4-matmul chain per slot.
  - x_attn is PE-transposed (identity trick) for the output projection; bo
    folded in via a ones-row matmul.
  - No max-subtraction in softmax: |scores| <~ 1.5 by construction.
"""

import sys

for _p in ("/opt/trn_rl_repo",):
    if _p not in sys.path:
        sys.path.insert(0, _p)

import numpy as np
import ml_dtypes

import concourse.bass as bass
import concourse.mybir as mybir
import concourse.tile as tile
from concourse import bacc
from concourse.bass_utils import run_bass_kernel_spmd
from concourse.masks import make_identity

B, L, F, H, D = 16, 1024, 512, 8, 64
NX, NY = 32, 32
NCORES = 8
BPC = B // NCORES  # batches per core
FP32 = mybir.dt.float32
BF16 = mybir.dt.bfloat16
FP16 = mybir.dt.float16
Exp = mybir.ActivationFunctionType.Exp
Identity = mybir.ActivationFunctionType.Identity
Mult = mybir.AluOpType.mult


def _build():
    nc = bacc.Bacc("TRN2", target_bir_lowering=False, debug=False)

    xqT_d = nc.dram_tensor("xqT", [BPC, 128, 4 * L], BF16, kind="ExternalInput").ap()
    xkvT_d = nc.dram_tensor("xkvT", [BPC, 128, 4 * L], BF16, kind="ExternalInput").ap()
    Wq_d = nc.dram_tensor("Wq", [128, 4 * F], BF16, kind="ExternalInput").ap()
    Wk_d = nc.dram_tensor("Wk", [128, 4 * F], BF16, kind="ExternalInput").ap()
    Wv_d = nc.dram_tensor("Wv", [128, 4 * F], BF16, kind="ExternalInput").ap()
    Wo_d = nc.dram_tensor("Wo", [128, 4 * F], BF16, kind="ExternalInput").ap()
    bq_d = nc.dram_tensor("bq", [F], FP32, kind="ExternalInput").ap()
    bk_d = nc.dram_tensor("bk", [F], FP32, kind="ExternalInput").ap()
    bv_d = nc.dram_tensor("bv", [128, F], BF16, kind="ExternalInput").ap()
    bo_d = nc.dram_tensor("bo", [128, F], BF16, kind="ExternalInput").ap()
    biasT_d = nc.dram_tensor("biasT", [H, 2, 128, 4 * L], FP16, kind="ExternalInput").ap()
    ones_d = nc.dram_tensor("ones", [128, 128], BF16, kind="ExternalInput").ap()
    out_d = nc.dram_tensor("out", [BPC, L, F], BF16, kind="ExternalOutput").ap()

    with tile.TileContext(nc) as tc:
        with (
            tc.tile_pool(name="const", bufs=1) as cpool,
            tc.tile_pool(name="xin", bufs=2) as xpool,
            tc.tile_pool(name="qkv", bufs=2) as qpool,
            tc.tile_pool(name="bias", bufs=3) as bpool,
            tc.tile_pool(name="work", bufs=2) as wpool,
            tc.tile_pool(name="exp", bufs=18) as epool,
            tc.tile_pool(name="es", bufs=3) as espool,
            tc.tile_pool(name="psA", bufs=3, space="PSUM") as psA,
            tc.tile_pool(name="psU", bufs=2, space="PSUM") as psU,
        ):
            # ---- constants ----
            Wv_s = cpool.tile([128, 4 * F], BF16, tag="Wv")
            Wq_s = cpool.tile([128, 4 * F], BF16, tag="Wq")
            Wk_s = cpool.tile([128, 4 * F], BF16, tag="Wk")
            Wo_s = cpool.tile([128, 4 * F], BF16, tag="Wo")

            bias_tiles = {}

            def stage_bias(h):
                tiles = []
                for hh in range(2):
                    bt = bpool.tile([128, 4 * L], FP16, tag="bias")
                    nc.sync.dma_start(out=bt[:], in_=biasT_d[h, hh])
                    tiles.append(bt)
                bias_tiles[h] = tiles

            stage_bias(0)
            nc.sync.dma_start(out=Wv_s[:], in_=Wv_d)
            nc.sync.dma_start(out=Wq_s[:], in_=Wq_d)
            nc.sync.dma_start(out=Wk_s[:], in_=Wk_d)
            ones_s = cpool.tile([128, 128], BF16, tag="ones")
            nc.sync.dma_start(out=ones_s[:], in_=ones_d)
            bv_s = cpool.tile([128, F], BF16, tag="bv")
            nc.sync.dma_start(out=bv_s[:], in_=bv_d)
            bq_s = cpool.tile([128, 4], FP32, tag="bq")
            bk_s = cpool.tile([128, 4], FP32, tag="bk")
            for b_s, b_d in ((bq_s, bq_d), (bk_s, bk_d)):
                nc.sync.dma_start(out=b_s[:], in_=b_d.rearrange("(c p) -> p c", p=128))

            qT, kT, vA, xan, xatT, xq, xkv = [], [], [], [], [], [], []
            for b in range(BPC):
                xq_t = xpool.tile([128, 4 * L], BF16, tag="xq")
                xkv_t = xpool.tile([128, 4 * L], BF16, tag="xkv")
                nc.sync.dma_start(out=xkv_t[:], in_=xkvT_d[b])
                nc.sync.dma_start(out=xq_t[:], in_=xqT_d[b])
                xq.append(xq_t)
                xkv.append(xkv_t)
                qT_t = qpool.tile([128, 4 * L], BF16, tag="qT")
                kT_t = qpool.tile([128, 4 * L], BF16, tag="kT")
                vA_t = qpool.tile([128, 8 * 8 * 65], FP16, tag="vA")
                xan_t = qpool.tile([128, 8 * F], BF16, tag="xan")
                xatT_t = qpool.tile([128, 4 * L], BF16, tag="xatT")
                qT.append(qT_t)
                kT.append(kT_t)
                vA.append(vA_t)
                xan.append(xan_t)
                xatT.append(xatT_t)
            stage_bias(1)
            load_late = [(Wo_s, Wo_d)]
            bo_s = cpool.tile([128, F], BF16, tag="bo")
            ident = cpool.tile([128, 128], BF16, tag="ident")
            make_identity(nc, ident[:])
            nc.sync.dma_start(out=Wo_s[:], in_=Wo_d)
            nc.sync.dma_start(out=bo_s[:], in_=bo_d)

            # ---- background PE work generators ----
            def v_proj_steps(b):
                # ones column for the softmax denominator accumulation
                nc.gpsimd.memset(
                    vA[b][:].rearrange("p (t h w) -> p t h w", t=8, h=8)[:, :, :, 64:65],
                    1.0,
                )
                # v natural (+bv via ones-row matmul): xT stationary, Wv moving
                for lt in range(8):
                    pv = psA.tile([128, 512], FP32, tag="ps")
                    for kc in range(4):
                        nc.tensor.matmul(
                            pv[:],
                            xkv[b][:, kc * L + lt * 128 : kc * L + (lt + 1) * 128],
                            Wv_s[:, kc * F : (kc + 1) * F],
                            start=(kc == 0),
                            stop=False,
                        )
                    nc.tensor.matmul(pv[:], ones_s[:], bv_s[:], start=False, stop=True)
                    nc.scalar.activation(
                        vA[b][:, lt * 520 : (lt + 1) * 520].rearrange(
                            "p (h w) -> p h w", h=8
                        )[:, :, 0:64],
                        pv[:].rearrange("p (h w) -> p h w", h=8),
                        Identity,
                        bias=0.0,
                    )
                    yield

            def qk_proj_steps(fo, b):
                # qT/kT transposed [fout, L]; head pair stacked on partitions
                for which, w_s, b_s, x_t, dst in (
                    ("q", Wq_s, bq_s, xq[b], qT[b]),
                    ("k", Wk_s, bk_s, xkv[b], kT[b]),
                ):
                    pq = psA.tile([128, 2 * 512], FP32, tag="ps")
                    for lc in range(2):
                        for kc in range(4):
                            nc.tensor.matmul(
                                pq[:, lc * 512 : (lc + 1) * 512],
                                w_s[:, kc * F + fo * 128 : kc * F + (fo + 1) * 128],
                                x_t[:, kc * L + lc * 512 : kc * L + (lc + 1) * 512],
                                start=(kc == 0),
                                stop=(kc == 3),
                            )
                        if lc == 0:
                            yield
                    nc.vector.tensor_scalar_add(
                        dst[:, fo * L : (fo + 1) * L], pq[:], b_s[:, fo : fo + 1]
                    )
                    yield

            def emit_c_steps(b):
                # transpose x_attn (identity trick) + output projection (+bo)
                for c in range(4):
                    for qt in range(8):
                        pt = psA.tile([128, 512], BF16, tag="ps")
                        nc.tensor.transpose(
                            pt[:, 0:128],
                            xan[b][:, qt * F + c * 128 : qt * F + (c + 1) * 128],
                            ident[:],
                        )
                        nc.vector.tensor_copy(
                            xatT[b][:, c * L + qt * 128 : c * L + (qt + 1) * 128],
                            pt[:, 0:128],
                        )
                        if qt % 4 == 3:
                            yield
                for lt in range(8):
                    po = psA.tile([128, 512], FP32, tag="ps")
                    for c in range(4):
                        nc.tensor.matmul(
                            po[:],
                            xatT[b][:, c * L + lt * 128 : c * L + (lt + 1) * 128],
                            Wo_s[:, c * F : (c + 1) * F],
                            start=(c == 0),
                            stop=False,
                        )
                    nc.tensor.matmul(po[:], ones_s[:], bo_s[:], start=False, stop=True)
                    os_t = wpool.tile([128, 512], BF16, tag="os")
                    nc.scalar.copy(os_t[:], po[:])
                    nc.sync.dma_start(
                        out=out_d[b, lt * 128 : (lt + 1) * 128, :], in_=os_t[:]
                    )
                    yield

            bg = []

            def bg_step(n=1):
                for _ in range(n):
                    while bg:
                        try:
                            next(bg[0])
                            break
                        except StopIteration:
                            bg.pop(0)
                    else:
                        return

            # ---- window-pipelined phase B ----
            # window w: scores/exp/bias-mult for (h, b) = (w//2, w%2);
            # attn@v (+ per-qt softmax normalize) for the previous pair.
            for _ in qk_proj_steps(0, 0):
                pass
            bg.append(qk_proj_steps(0, 1))
            bg.append(v_proj_steps(0))
            bg.append(v_proj_steps(1))
            pending_proj = {1: (1, 0), 2: (1, 1), 5: (2, 0), 6: (2, 1),
                            9: (3, 0), 10: (3, 1)}

            def attnv_qt(h, b, exs, qt):
                U = psU.tile([128, 65], FP32, tag="u")
                for kt in range(8):
                    nc.tensor.matmul(
                        U[:],
                        exs[kt][:, qt * 128 : (qt + 1) * 128],
                        vA[b][:, kt * 520 + h * 65 : kt * 520 + (h + 1) * 65],
                        start=(kt == 0),
                        stop=(kt == 7),
                    )
                rc = wpool.tile([128, 1], FP32, tag="rc")
                nc.vector.reciprocal(rc[:], U[:, 64:65])
                nc.vector.tensor_scalar(
                    xan[b][:, qt * F + h * 64 : qt * F + (h + 1) * 64],
                    U[:, 0:64],
                    rc[:],
                    None,
                    op0=Mult,
                )

            prev = None  # (h, b, [ex tiles]) of the pair being drained
            for w in range(2 * H):
                h, b = w // 2, w % 2
                hp = (h % 2) * 64
                hc = (h // 2) * L
                if w in pending_proj:
                    bg.append(qk_proj_steps(*pending_proj[w]))
                if b == 0 and h >= 1 and h + 1 < H:
                    stage_bias(h + 1)
                exs = []
                for kt in range(8):
                    ps = psA.tile([128, 2 * 512], FP32, tag="ps")
                    for qc in range(2):
                        nc.tensor.matmul(
                            ps[:, qc * 512 : (qc + 1) * 512],
                            kT[b][hp : hp + 64, hc + kt * 128 : hc + (kt + 1) * 128],
                            qT[b][hp : hp + 64, hc + qc * 512 : hc + (qc + 1) * 512],
                            start=True,
                            stop=True,
                        )
                    es = espool.tile([128, 2 * 512], FP16, tag="es")
                    nc.scalar.activation(es[:], ps[:], Exp)
                    ex = epool.tile([128, 2 * 512], FP16, tag="ex")
                    nc.vector.tensor_tensor(
                        ex[:],
                        es[:],
                        bias_tiles[h][kt // 4][:, (kt % 4) * L : (kt % 4 + 1) * L],
                        Mult,
                    )
                    exs.append(ex)
                    if prev is not None and kt >= 1:
                        attnv_qt(prev[0], prev[1], prev[2], kt - 1)
                    bg_step(2 if w <= 1 else 1)
                if prev is not None:
                    attnv_qt(prev[0], prev[1], prev[2], 7)
                    if prev[0] == H - 1:
                        bg.append(emit_c_steps(prev[1]))
                prev = (h, b, exs)

            # drain: attn@v for the last pair (h=H-1, b=1)
            for qt in range(8):
                attnv_qt(prev[0], prev[1], prev[2], qt)
                bg_step(2)
            bg.append(emit_c_steps(prev[1]))
            while bg:
                bg_step()

    nc.compile()
    return nc


_NC = None


def _get_nc():
    global _NC
    if _NC is None:
        _NC = _build()
    return _NC


def _prep_in_maps(inputs):
    bf16 = ml_dtypes.bfloat16
    xq = np.asarray(inputs["inputs_q"], dtype=np.float32)
    xkv = np.asarray(inputs["inputs_kv"], dtype=np.float32)
    Wq = np.asarray(inputs["Wq"], dtype=np.float32) * 0.125
    bq = np.asarray(inputs["bq"], dtype=np.float32) * 0.125
    Wk = np.asarray(inputs["Wk"], dtype=np.float32)
    bk = np.asarray(inputs["bk"], dtype=np.float32)
    Wv = np.asarray(inputs["Wv"], dtype=np.float32)
    Wo = np.asarray(inputs["Wo"], dtype=np.float32)
    bv_pad = np.zeros((128, F), dtype=np.float32)
    bv_pad[0] = np.asarray(inputs["bv"], dtype=np.float32)
    bo_pad = np.zeros((128, F), dtype=np.float32)
    bo_pad[0] = np.asarray(inputs["bo"], dtype=np.float32)
    onesrow = np.zeros((128, 128), dtype=np.float32)
    onesrow[0] = 1.0
    toe = np.asarray(inputs["toeplitz"], dtype=np.float32)

    def pack_x(x):
        # [B, L, F] -> xT [B, F, L] -> partition-major [B, 128, 4*L]
        xT = x.transpose(0, 2, 1).reshape(B, 4, 128, L)
        return np.ascontiguousarray(xT.transpose(0, 2, 1, 3)).reshape(
            B, 128, 4 * L
        ).astype(bf16)

    def pack_w(w):
        # [F, F] -> partition-major [128, 4*F]
        return np.ascontiguousarray(
            w.reshape(4, 128, F).transpose(1, 0, 2)
        ).reshape(128, 4 * F).astype(bf16)

    xqT = pack_x(xq)
    xkvT = pack_x(xkv)
    WqP, WkP, WvP, WoP = pack_w(Wq), pack_w(Wk), pack_w(Wv), pack_w(Wo)

    coords = np.arange(L)
    xi, yi = coords // NY, coords % NY
    dx = xi[:, None] - xi[None, :] + NX
    dy = yi[:, None] - yi[None, :] + NY
    idx = dx * (2 * NY) + dy  # [L(q), L(k)]
    bias = toe[:, idx]  # [H, L(q), L(k)]
    biasT = np.exp(bias.transpose(0, 2, 1))  # [H, L(k), L(q)]
    # partition-major [H, 2, 128, 4*L]
    biasT = np.ascontiguousarray(
        biasT.reshape(H, 2, 4, 128, L).transpose(0, 1, 3, 2, 4)
    ).reshape(H, 2, 128, 4 * L).astype(np.float16)

    in_maps = []
    for i in range(NCORES):
        sl = slice(i * BPC, (i + 1) * BPC)
        in_maps.append(
            {
                "xqT": np.ascontiguousarray(xqT[sl]),
                "xkvT": np.ascontiguousarray(xkvT[sl]),
                "Wq": WqP, "Wk": WkP, "Wv": WvP, "Wo": WoP,
                "bq": bq, "bk": bk,
                "bv": bv_pad.astype(bf16), "bo": bo_pad.astype(bf16),
                "biasT": biasT,
                "ones": onesrow.astype(bf16),
            }
        )
    return in_maps


def _run(inputs, trace=False):
    from concourse.bass_interp import get_hw_module

    nc = _get_nc()
    in_maps = _prep_in_maps(inputs)
    old_m = nc.m
    nc.m = get_hw_module(nc.m)
    try:
        res = run_bass_kernel_spmd(
            nc, in_maps, core_ids=list(range(NCORES)), trace=trace
        )
    finally:
        nc.m = old_m
    out = np.concatenate(
        [np.asarray(r["out"], dtype=np.float32) for r in res.results], axis=0
    )  # [B, L, F]
    return out.reshape(B, L, H, D), res


def kernel(**inputs) -> np.ndarray:
    out, _ = _run(inputs, trace=False)
    return out
